# revision 1
# baseline (speedup 1.0000x reference)
"""GAT message-passing kernel for 8 Trainium2 NeuronCores.

Strategy (dst-sharded graph parallel):
  - Nodes are padded to 50176 and split into 8 contiguous shards of 6272
    (49 blocks of 128 dst nodes per core).  Each core owns the edges whose
    destination falls in its shard (edges sorted by dst block on host).
  - Every core computes the full projection h = x @ W (plus per-node
    attention halves a_src/a_dst folded into the same matmul) and writes a
    per-node row table [p-slot | a_src | h] to its HBM.
  - Edge phase: per 128-edge tile, rows are fetched with a SWDGE dma_gather
    (512 B rows, int16 indices; the node table is split <32768 / >=32768 so
    indices fit int16). Per-edge a_dst comes from a PE matmul of a
    host-prepped one-hot selT tile (fp8 -- exact for 0/1 -- streamed
    contiguously from HBM) against an SBUF-resident f16 per-block a_dst
    table -- no second gather. Attention
    p = exp(leaky(a_src+a_dst)) is written into the gathered tile's p-slot,
    messages are scaled in place, and a one-hot selection matrix (built from
    iota==dst_local) is used as the stationary operand of a single PE matmul
    per tile that accumulates both the weighted messages and the softmax
    denominator in PSUM per 128-node block.
  - Host does index/layout prep only (edge sorting, padding, int16 index
    streams); all floating-point math runs on device.
"""

import sys

sys.path.insert(0, "/opt/trn_rl_repo")

import numpy as np

N = 50000
E_IN = 600000
F = 128          # feature dim (both in and out)
H = 4
D = 32
NEG = 0.2
NCORES = 8
BLK = 128
NB = 49                  # blocks per core
OWN = NB * BLK           # 6272
NPAD = NCORES * OWN      # 50176
NT = NPAD // 128         # 392 projection tiles
LOW = 32768              # src-index split for int16 gather indices
HI_ROWS = NPAD - LOW     # 17408
G = 4                    # blocks per gather group
PHASE = 3                # debug: 1=projection only, 2=+gathers, 3=full

_CACHE = {}


def _wrap_idx(flat):
    """int16 stream -> [128, len/16] SBUF layout (wrapped in 16 partitions,
    replicated across the 8 gpsimd cores)."""
    flat = np.ascontiguousarray(flat, dtype=np.int16)
    assert flat.size % 16 == 0
    w = flat.reshape(-1, 16).T              # [16, slots]
    out = np.empty((128, flat.size // 16), np.int16)
    for g in range(8):
        out[16 * g:16 * (g + 1)] = w
    return out


def _host_prep(x, edge_index, W, att_src, att_dst, bias):
    import ml_dtypes
    global _f8
    _f8 = ml_dtypes.float8_e4m3
    f16 = np.float16
    src0 = np.asarray(edge_index[0], dtype=np.int64)
    dst0 = np.asarray(edge_index[1], dtype=np.int64)
    loops = np.arange(NPAD, dtype=np.int64)
    src = np.concatenate([src0, loops])
    dst = np.concatenate([dst0, loops])

    core = dst // OWN
    blk = (dst % OWN) // BLK
    dl = (dst % BLK).astype(np.float32)
    is_hi = (src >= LOW).astype(np.int64)
    key = (core * NB + blk) * 2 + is_hi
    order = np.argsort(key, kind="stable")
    src_s = src[order]
    dl_s = dl[order]
    key_s = key[order]
    dst_own_s = (dst % OWN)[order]

    counts = np.bincount(key_s, minlength=NCORES * NB * 2)
    tiles = -(-counts // 128)
    T_LO = int(tiles[0::2].max())
    T_HI = int(tiles[1::2].max())
    TPB = T_LO + T_HI
    NTILES = NB * TPB

    starts = np.zeros(NCORES * NB * 2 + 1, np.int64)
    np.cumsum(counts, out=starts[1:])
    rank = np.arange(src_s.size, dtype=np.int64) - starts[key_s]
    c_s = key_s // (2 * NB)
    b_s = (key_s // 2) % NB
    hi_s = key_s & 1
    tau = np.where(hi_s == 0, rank // 128, T_LO + rank // 128)
    J = b_s * TPB + tau                      # tile index within core
    part = rank % 128

    # per-core padded streams
    idx_lo = np.zeros((NCORES, NB * T_LO * 128), np.int16)
    idx_hi = np.zeros((NCORES, NB * T_HI * 128), np.int16)
    dstloc = np.full((NCORES, 128, NTILES), -1.0, np.float16)
    selT = np.zeros((NCORES, 128, NTILES, 128), _f8)

    lo_m = hi_s == 0
    pos_lo = (b_s[lo_m] * T_LO + rank[lo_m] // 128) * 128 + part[lo_m]
    idx_lo[c_s[lo_m], pos_lo] = src_s[lo_m].astype(np.int16)
    hi_m = ~lo_m
    pos_hi = (b_s[hi_m] * T_HI + rank[hi_m] // 128) * 128 + part[hi_m]
    idx_hi[c_s[hi_m], pos_hi] = (src_s[hi_m] - LOW).astype(np.int16)
    dstloc[c_s, part, J] = dl_s
    selT[c_s, dl_s.astype(np.int64), J, part] = 1.0

    # shared constants
    xT = np.zeros((F, NPAD), np.float16)
    xT[:, :N] = np.asarray(x, np.float32).T.astype(np.float16)
    Wf = np.ascontiguousarray(np.asarray(W, np.float32))
    WT = np.ascontiguousarray(Wf.T)
    Asrc = np.zeros((F, H), np.float32)
    Adst = np.zeros((F, H), np.float32)
    for hh in range(H):
        Asrc[hh * D:(hh + 1) * D, hh] = np.asarray(att_src, np.float32)[hh]
        Adst[hh * D:(hh + 1) * D, hh] = np.asarray(att_dst, np.float32)[hh]
    bias_rep = np.ascontiguousarray(
        np.broadcast_to(np.asarray(bias, np.float32), (128, F)))
    io_rep = np.ascontiguousarray(np.broadcast_to(
        np.repeat(np.arange(128, dtype=f16), TPB)[None, :], (128, 128 * TPB)))

    in_maps = []
    for c in range(NCORES):
        in_maps.append({
            "xT": xT,
            "xT_own": np.ascontiguousarray(xT[:, c * OWN:(c + 1) * OWN]),
            "W": Wf,
            "WT": WT,
            "Asrc": Asrc,
            "Adst": Adst,
            "bias_rep": bias_rep,
            "io_rep": io_rep,
            "idx_lo": _wrap_idx(idx_lo[c]),
            "idx_hi": _wrap_idx(idx_hi[c]),
            "selT": np.ascontiguousarray(selT[c].reshape(128, NTILES * 128)),
            "dstloc": dstloc[c],
        })
    return in_maps, T_LO, T_HI


def _build_program(T_LO, T_HI):
    import concourse.bacc as bacc
    import concourse.mybir as mybir
    import concourse.tile as tile

    F16 = mybir.dt.float16
    F32 = mybir.dt.float32
    F8 = mybir.dt.float8e4
    I16 = mybir.dt.int16
    AOP = mybir.AluOpType
    ACT = mybir.ActivationFunctionType

    TPB = T_LO + T_HI
    NTILES = NB * TPB

    nc = bacc.Bacc("TRN2", target_bir_lowering=False)

    xT = nc.dram_tensor("xT", [F, NPAD], F16, kind="ExternalInput")
    xT_own = nc.dram_tensor("xT_own", [F, OWN], F16, kind="ExternalInput")
    W_d = nc.dram_tensor("W", [F, F], F32, kind="ExternalInput")
    WT_d = nc.dram_tensor("WT", [F, F], F32, kind="ExternalInput")
    As_d = nc.dram_tensor("Asrc", [F, H], F32, kind="ExternalInput")
    Ad_d = nc.dram_tensor("Adst", [F, H], F32, kind="ExternalInput")
    bias_d = nc.dram_tensor("bias_rep", [128, F], F32, kind="ExternalInput")
    iorep_d = nc.dram_tensor("io_rep", [128, 128 * TPB], F16, kind="ExternalInput")
    idxlo_d = nc.dram_tensor("idx_lo", [128, NB * T_LO * 8], I16, kind="ExternalInput")
    idxhi_d = nc.dram_tensor("idx_hi", [128, NB * T_HI * 8], I16, kind="ExternalInput")
    selT_d = nc.dram_tensor("selT", [128, NTILES * 128], F8, kind="ExternalInput")
    dstloc_d = nc.dram_tensor("dstloc", [128, NTILES], F16, kind="ExternalInput")

    htab = nc.dram_tensor("htab", [NPAD, 256], F16)      # row: [p|a_src|h|pad]
    out_d = nc.dram_tensor("out", [OWN, F], F32, kind="ExternalOutput")

    with tile.TileContext(nc) as tc:
        with tc.tile_pool(name="const", bufs=1) as cp:
            W_t = cp.tile([F, F], F32)
            nc.sync.dma_start(out=W_t[:], in_=W_d[:])
            WT_t = cp.tile([F, F], F32)
            nc.sync.dma_start(out=WT_t[:], in_=WT_d[:])
            As_t = cp.tile([F, H], F32)
            nc.sync.dma_start(out=As_t[:], in_=As_d[:])
            Ad_t = cp.tile([F, H], F32)
            nc.sync.dma_start(out=Ad_t[:], in_=Ad_d[:])
            bias_t = cp.tile([128, F], F32)
            nc.sync.dma_start(out=bias_t[:], in_=bias_d[:])
            io_t = cp.tile([128, 128 * TPB], F16)
            nc.sync.dma_start(out=io_t[:], in_=iorep_d[:])
            idxlo_t = cp.tile([128, NB * T_LO * 8], I16)
            nc.sync.dma_start(out=idxlo_t[:], in_=idxlo_d[:])
            idxhi_t = cp.tile([128, NB * T_HI * 8], I16)
            nc.sync.dma_start(out=idxhi_t[:], in_=idxhi_d[:])
            dstloc_t = cp.tile([128, NTILES], F16)
            nc.sync.dma_start(out=dstloc_t[:], in_=dstloc_d[:])
            adall = cp.tile([128, NB, 4], F16)   # own-shard a_dst per block

            # Wcat = [W@Asrc | W | W@Adst]  -> proj psum: [a_src | h | a_dst]
            wcat = cp.tile([F, 136], F32)
            with tc.tile_pool(name="wps", bufs=1, space="PSUM") as wps:
                wa = wps.tile([F, 8], F32)
                nc.tensor.matmul(wa[:, 0:4], lhsT=WT_t[:], rhs=As_t[:],
                                 start=True, stop=True)
                nc.tensor.matmul(wa[:, 4:8], lhsT=WT_t[:], rhs=Ad_t[:],
                                 start=True, stop=True)
                nc.vector.tensor_copy(out=wcat[:, 0:4], in_=wa[:, 0:4])
                nc.vector.tensor_copy(out=wcat[:, 132:136], in_=wa[:, 4:8])
                nc.any.tensor_copy(out=wcat[:, 4:132], in_=W_t[:])
            wcat16 = cp.tile([F, 136], F16)
            nc.any.tensor_copy(out=wcat16[:], in_=wcat[:])

            # ---------------- phase 1: projection ----------------
            PB = 16
            with tc.tile_pool(name="p1", bufs=4) as p1, \
                 tc.tile_pool(name="p1ps", bufs=6, space="PSUM") as p1ps, \
                 tc.tile_pool(name="p1ps2", bufs=2, space="PSUM") as p1ps2:
                for t0 in range(0, NT, PB):
                    nb_t = min(PB, NT - t0)
                    xt = p1.tile([128, PB, 128], F16, tag="xt")
                    nc.sync.dma_start(
                        out=xt[:, 0:nb_t, :],
                        in_=xT[:, t0 * 128:(t0 + nb_t) * 128])
                    row = p1.tile([128, PB, 136], F16, tag="row")
                    for k in range(nb_t):
                        ps = p1ps.tile([128, 136], F32, tag="pps")
                        nc.tensor.matmul(ps[:], lhsT=xt[:, k, :], rhs=wcat16[:],
                                         start=True, stop=True)
                        nc.any.tensor_copy(out=row[:, k, 4:136], in_=ps[:, 0:132])
                    dst = htab[t0 * 128:(t0 + nb_t) * 128, 4:136].rearrange(
                        "(k p) c -> p k c", k=nb_t)
                    nc.sync.dma_start(out=dst, in_=row[:, 0:nb_t, 4:136])
                for t0 in range(0, NB, PB):
                    nb_t = min(PB, NB - t0)
                    xo = p1.tile([128, PB, 128], F16, tag="xt")
                    nc.sync.dma_start(
                        out=xo[:, 0:nb_t, :],
                        in_=xT_own[:, t0 * 128:(t0 + nb_t) * 128])
                    for k in range(nb_t):
                        ps2 = p1ps2.tile([128, 4], F32, tag="pps2")
                        nc.tensor.matmul(ps2[:], lhsT=xo[:, k, :],
                                         rhs=wcat16[:, 132:136],
                                         start=True, stop=True)
                        nc.any.tensor_copy(out=adall[:, t0 + k, :], in_=ps2[:])

            # ---------------- phase 2: edge processing ----------------
            if PHASE < 2:
                with tc.tile_pool(name="dbg", bufs=2) as dbg:
                    for q in range(NB):
                        z = dbg.tile([128, F], F16, tag="z")
                        nc.sync.dma_start(
                            out=z[:], in_=htab[q * 128:(q + 1) * 128, 0:128])
                        zo = dbg.tile([128, F], F32, tag="zo")
                        nc.any.tensor_copy(out=zo[:], in_=z[:])
                        nc.sync.dma_start(
                            out=out_d[q * 128:(q + 1) * 128, :], in_=zo[:])
            else:
              with tc.tile_pool(name="ep", bufs=3) as ep, \
                 tc.tile_pool(name="selp", bufs=4) as selp, \
                 tc.tile_pool(name="op", bufs=6) as op_, \
                 tc.tile_pool(name="eps", bufs=6, space="PSUM") as eps, \
                 tc.tile_pool(name="adps_p", bufs=2, space="PSUM") as adps_p:
                ngrp = -(-NB // G)
                for g in range(ngrp):
                    b0 = G * g
                    nb = min(G, NB - b0)
                    n_lo = nb * T_LO
                    n_hi = nb * T_HI
                    ntg = nb * TPB

                    # single_packet=True is the fast Q7 DGE path, but packets
                    # cap at 64 descriptors per engine -> at most 1024 idxs
                    # (8 tiles) per call.
                    def chunked_gather(out_tile, n_tiles, in_ap, idx_tile,
                                       slot0, elem):
                        done = 0
                        while done < n_tiles:
                            k = min(8, n_tiles - done)
                            nc.gpsimd.dma_gather(
                                out_ap=out_tile[:, done:done + k, :],
                                in_ap=in_ap,
                                idxs_ap=idx_tile[:, slot0 + done * 8:
                                                 slot0 + (done + k) * 8],
                                num_idxs=k * 128, num_idxs_reg=k * 128,
                                elem_size=elem, single_packet=True)
                            done += k

                    hlo = ep.tile([128, G * T_LO, 256], F16, tag="hlo")
                    chunked_gather(hlo, n_lo, htab[0:LOW, :], idxlo_t,
                                   b0 * T_LO * 8, 256)
                    hhi = ep.tile([128, G * T_HI, 256], F16, tag="hhi")
                    chunked_gather(hhi, n_hi, htab[LOW:NPAD, :], idxhi_t,
                                   b0 * T_HI * 8, 256)
                    selT_g = ep.tile([128, G * TPB, 128], F8, tag="selTg")
                    nc.sync.dma_start(
                        out=selT_g[:, 0:ntg, :],
                        in_=selT_d[:, b0 * TPB * 128:(b0 * TPB + ntg) * 128])

                    if PHASE < 3:
                        # consume gathers trivially
                        for q in range(nb):
                            zo = op_.tile([128, F], F32, tag="o")
                            nc.any.tensor_copy(
                                out=zo[:], in_=hlo[:, q * T_LO, 0:128])
                            nc.any.tensor_add(
                                out=zo[:], in0=zo[:],
                                in1=selT_g[:, q * TPB, 0:128])
                            nc.any.tensor_add(
                                out=zo[:], in0=zo[:],
                                in1=hhi[:, q * T_HI, 0:128])
                            nc.sync.dma_start(
                                out=out_d[(b0 + q) * 128:(b0 + q + 1) * 128, :],
                                in_=zo[:])
                        continue

                    # a_dst per edge: adps[e, (q,t,h)] = selT^T @ adall[block]
                    adps = adps_p.tile([128, G * TPB, 4], F32, tag="adps")
                    for tau in range(ntg):
                        nc.tensor.matmul(adps[:, tau, :],
                                         lhsT=selT_g[:, tau, :],
                                         rhs=adall[:, b0 + tau // TPB, :],
                                         start=True, stop=True)
                    # logits = a_src + a_dst  (block-major [q, tau, h] layout)
                    lg = ep.tile([128, G * TPB, 4], F32, tag="lg")
                    lgq = lg[:, 0:ntg, :].rearrange("p (q t) h -> p q t h", q=nb)
                    adq = adps[:, 0:ntg, :].rearrange("p (q t) h -> p q t h", q=nb)
                    asl = hlo[:, 0:n_lo, 4:8].rearrange("p (q t) h -> p q t h", q=nb)
                    ash = hhi[:, 0:n_hi, 4:8].rearrange("p (q t) h -> p q t h", q=nb)
                    nc.any.tensor_add(out=lgq[:, :, 0:T_LO, :], in0=asl,
                                      in1=adq[:, :, 0:T_LO, :])
                    nc.any.tensor_add(out=lgq[:, :, T_LO:TPB, :], in0=ash,
                                      in1=adq[:, :, T_LO:TPB, :])
                    # leaky relu: max(0.2*x, x)
                    lk = ep.tile([128, G * TPB * 4], F32, tag="lk")
                    lgf = lg[:, 0:ntg, :].rearrange("p t h -> p (t h)")
                    nc.vector.scalar_tensor_tensor(
                        out=lk[:, 0:ntg * 4], in0=lgf, scalar=NEG, in1=lgf,
                        op0=AOP.mult, op1=AOP.max)
                    # exp -> p written into gathered tiles' p-slots
                    lkq = lk[:, 0:ntg * 4].rearrange("p (q t h) -> p q t h",
                                                     q=nb, t=TPB)
                    plo = hlo[:, 0:n_lo, 0:4].rearrange("p (q t) h -> p q t h", q=nb)
                    phi = hhi[:, 0:n_hi, 0:4].rearrange("p (q t) h -> p q t h", q=nb)
                    nc.scalar.activation(out=plo, in_=lkq[:, :, 0:T_LO, :],
                                         func=ACT.Exp)
                    nc.scalar.activation(out=phi, in_=lkq[:, :, T_LO:TPB, :],
                                         func=ACT.Exp)
                    # messages: h *= p (broadcast over feature groups)
                    for hx, n_x in ((hlo, n_lo), (hhi, n_hi)):
                        hv = hx[:, 0:n_x, 8:136].rearrange(
                            "p t (h d) -> p t h d", h=4)
                        pv = hx[:, 0:n_x, 0:4].rearrange(
                            "p t (h o) -> p t h o", o=1).to_broadcast(
                            [128, n_x, 4, 32])
                        nc.any.tensor_mul(out=hv, in0=hv, in1=pv)

                    # per-block aggregation; sel for all TPB tiles of a
                    # block built in one 2x-mode tensor_tensor:
                    # sel[p, j, t] = (io_rep[p, j, t] == dstloc[p, t])
                    for q in range(nb):
                        Jb = (b0 + q) * TPB
                        sel = selp.tile([128, 128, TPB], F16, tag="sel")
                        dl = dstloc_t[:, Jb:Jb + TPB].rearrange(
                            "p (o t) -> p o t", o=1).to_broadcast([128, 128, TPB])
                        nc.vector.tensor_tensor(out=sel[:], in0=io_t[:].rearrange(
                            "p (j t) -> p j t", j=128), in1=dl, op=AOP.is_equal)
                        bps = eps.tile([128, 136], F32, tag="bps")
                        for tau in range(TPB):
                            if tau < T_LO:
                                rhs = hlo[:, q * T_LO + tau, 0:136]
                            else:
                                rhs = hhi[:, q * T_HI + (tau - T_LO), 0:136]
                            nc.tensor.matmul(bps[:], lhsT=sel[:, :, tau], rhs=rhs,
                                             start=(tau == 0),
                                             stop=(tau == TPB - 1))
                        # psum: [denom 0:4 | junk 4:8 | feats 8:136]
                        rcp = op_.tile([128, 4], F32, tag="rcp")
                        nc.vector.reciprocal(rcp[:], bps[:, 0:4])
                        o = op_.tile([128, F], F32, tag="o")
                        ov = o[:].rearrange("p (h d) -> p h d", h=H)
                        rv = rcp[:].rearrange("p (h o) -> p h o", o=1).to_broadcast(
                            [128, H, D])
                        nc.vector.tensor_tensor(
                            out=ov,
                            in0=bps[:, 8:136].rearrange("p (h d) -> p h d", h=H),
                            in1=rv, op=AOP.mult)
                        nc.vector.tensor_tensor(out=o[:], in0=o[:],
                                                in1=bias_t[:], op=AOP.add)
                        nc.scalar.activation(out=o[:], in_=o[:], func=ACT.Tanh)
                        nc.sync.dma_start(
                            out=out_d[(b0 + q) * 128:(b0 + q + 1) * 128, :],
                            in_=o[:])

    nc.compile()
    return nc


def kernel(**inputs):
    x = inputs["x"]
    edge_index = inputs["edge_index"]
    W = inputs["W"]
    att_src = inputs["att_src"]
    att_dst = inputs["att_dst"]
    bias = inputs["bias"]
    assert x.shape == (N, F) and edge_index.shape == (2, E_IN)

    from concourse import bass_utils

    in_maps, T_LO, T_HI = _host_prep(x, edge_index, W, att_src, att_dst, bias)
    key = (T_LO, T_HI)
    if key not in _CACHE:
        _CACHE[key] = _build_program(T_LO, T_HI)
    nc = _CACHE[key]
    res = bass_utils.run_bass_kernel_spmd(nc, in_maps, core_ids=list(range(NCORES)))
    out = np.concatenate([res.results[c]["out"] for c in range(NCORES)], axis=0)
    return np.ascontiguousarray(out[:N]).astype(np.float32)



# revision 2
# speedup vs baseline: 68.0203x; 68.0203x over previous
"""GAT message-passing kernel for 8 Trainium2 NeuronCores.

Strategy (dst-sharded graph parallel):
  - Nodes are padded to 50176 and split into 8 contiguous shards of 6272
    (49 blocks of 128 dst nodes per core).  Each core owns the edges whose
    destination falls in its shard (edges sorted by dst block on host).
  - Every core computes the full projection h = x @ W (plus per-node
    attention halves a_src/a_dst folded into the same matmul) and writes a
    per-node row table [p-slot | a_src | h] to its HBM.
  - Edge phase: per 128-edge tile, rows are fetched with a SWDGE dma_gather
    (512 B rows, int16 indices; the node table is split <32768 / >=32768 so
    indices fit int16). Per-edge a_dst comes from a PE matmul of a
    host-prepped one-hot selT tile (fp8 -- exact for 0/1 -- streamed
    contiguously from HBM) against an SBUF-resident f16 per-block a_dst
    table -- no second gather. Attention
    p = exp(leaky(a_src+a_dst)) is written into the gathered tile's p-slot,
    messages are scaled in place, and a one-hot selection matrix (built from
    iota==dst_local) is used as the stationary operand of a single PE matmul
    per tile that accumulates both the weighted messages and the softmax
    denominator in PSUM per 128-node block.
  - Host does index/layout prep only (edge sorting, padding, int16 index
    streams); all floating-point math runs on device.
"""

import sys

sys.path.insert(0, "/opt/trn_rl_repo")

import numpy as np

N = 50000
E_IN = 600000
F = 128          # feature dim (both in and out)
H = 4
D = 32
NEG = 0.2
NCORES = 8
BLK = 128
NB = 49                  # blocks per core
OWN = NB * BLK           # 6272
NPAD = NCORES * OWN      # 50176
NT = NPAD // 128         # 392 projection tiles
LOW = 32768              # src-index split for int16 gather indices
HI_ROWS = NPAD - LOW     # 17408
G = 4                    # blocks per gather group
PHASE = 3                # debug: 1=projection only, 2=+gathers, 3=full

_CACHE = {}


def _wrap_idx(flat):
    """int16 stream -> [128, len/16] SBUF layout (wrapped in 16 partitions,
    replicated across the 8 gpsimd cores)."""
    flat = np.ascontiguousarray(flat, dtype=np.int16)
    assert flat.size % 16 == 0
    w = flat.reshape(-1, 16).T              # [16, slots]
    out = np.empty((128, flat.size // 16), np.int16)
    for g in range(8):
        out[16 * g:16 * (g + 1)] = w
    return out


def _host_prep(x, edge_index, W, att_src, att_dst, bias):
    import ml_dtypes
    global _f8
    _f8 = ml_dtypes.float8_e4m3
    f16 = np.float16
    src0 = np.asarray(edge_index[0], dtype=np.int64)
    dst0 = np.asarray(edge_index[1], dtype=np.int64)
    loops = np.arange(NPAD, dtype=np.int64)
    src = np.concatenate([src0, loops])
    dst = np.concatenate([dst0, loops])

    core = dst // OWN
    blk = (dst % OWN) // BLK
    dl = (dst % BLK).astype(np.float32)
    is_hi = (src >= LOW).astype(np.int64)
    key = (core * NB + blk) * 2 + is_hi
    order = np.argsort(key, kind="stable")
    src_s = src[order]
    dl_s = dl[order]
    key_s = key[order]
    dst_own_s = (dst % OWN)[order]

    counts = np.bincount(key_s, minlength=NCORES * NB * 2)
    tiles = -(-counts // 128)
    T_LO = int(tiles[0::2].max())
    T_HI = int(tiles[1::2].max())
    TPB = T_LO + T_HI
    NTILES = NB * TPB

    starts = np.zeros(NCORES * NB * 2 + 1, np.int64)
    np.cumsum(counts, out=starts[1:])
    rank = np.arange(src_s.size, dtype=np.int64) - starts[key_s]
    c_s = key_s // (2 * NB)
    b_s = (key_s // 2) % NB
    hi_s = key_s & 1
    tau = np.where(hi_s == 0, rank // 128, T_LO + rank // 128)
    J = b_s * TPB + tau                      # tile index within core
    part = rank % 128

    # per-core padded streams
    idx_lo = np.zeros((NCORES, NB * T_LO * 128), np.int16)
    idx_hi = np.zeros((NCORES, NB * T_HI * 128), np.int16)
    dstloc = np.full((NCORES, 128, NTILES), -1.0, np.float16)
    selT = np.zeros((NCORES, 128, NTILES, 128), _f8)

    lo_m = hi_s == 0
    pos_lo = (b_s[lo_m] * T_LO + rank[lo_m] // 128) * 128 + part[lo_m]
    idx_lo[c_s[lo_m], pos_lo] = src_s[lo_m].astype(np.int16)
    hi_m = ~lo_m
    pos_hi = (b_s[hi_m] * T_HI + rank[hi_m] // 128) * 128 + part[hi_m]
    idx_hi[c_s[hi_m], pos_hi] = (src_s[hi_m] - LOW).astype(np.int16)
    dstloc[c_s, part, J] = dl_s
    selT[c_s, dl_s.astype(np.int64), J, part] = 1.0

    # shared constants
    xT = np.zeros((F, NPAD), np.float16)
    xT[:, :N] = np.asarray(x, np.float32).T.astype(np.float16)
    Wf = np.ascontiguousarray(np.asarray(W, np.float32))
    WT = np.ascontiguousarray(Wf.T)
    Asrc = np.zeros((F, H), np.float32)
    Adst = np.zeros((F, H), np.float32)
    for hh in range(H):
        Asrc[hh * D:(hh + 1) * D, hh] = np.asarray(att_src, np.float32)[hh]
        Adst[hh * D:(hh + 1) * D, hh] = np.asarray(att_dst, np.float32)[hh]
    bias_rep = np.ascontiguousarray(
        np.broadcast_to(np.asarray(bias, np.float32), (128, F)))
    io_rep = np.ascontiguousarray(np.broadcast_to(
        np.repeat(np.arange(128, dtype=f16), TPB)[None, :], (128, 128 * TPB)))

    in_maps = []
    for c in range(NCORES):
        in_maps.append({
            "xT": xT,
            "xT_own": np.ascontiguousarray(xT[:, c * OWN:(c + 1) * OWN]),
            "W": Wf,
            "WT": WT,
            "Asrc": Asrc,
            "Adst": Adst,
            "bias_rep": bias_rep,
            "io_rep": io_rep,
            "idx_lo": _wrap_idx(idx_lo[c]),
            "idx_hi": _wrap_idx(idx_hi[c]),
            "selT": np.ascontiguousarray(selT[c].reshape(128, NTILES * 128)),
            "dstloc": dstloc[c],
        })
    return in_maps, T_LO, T_HI


def _build_program(T_LO, T_HI):
    import concourse.bacc as bacc
    import concourse.mybir as mybir
    import concourse.tile as tile

    F16 = mybir.dt.float16
    F32 = mybir.dt.float32
    F8 = mybir.dt.float8e4
    I16 = mybir.dt.int16
    AOP = mybir.AluOpType
    ACT = mybir.ActivationFunctionType

    TPB = T_LO + T_HI
    NTILES = NB * TPB

    nc = bacc.Bacc("TRN2", target_bir_lowering=False)

    xT = nc.dram_tensor("xT", [F, NPAD], F16, kind="ExternalInput")
    xT_own = nc.dram_tensor("xT_own", [F, OWN], F16, kind="ExternalInput")
    W_d = nc.dram_tensor("W", [F, F], F32, kind="ExternalInput")
    WT_d = nc.dram_tensor("WT", [F, F], F32, kind="ExternalInput")
    As_d = nc.dram_tensor("Asrc", [F, H], F32, kind="ExternalInput")
    Ad_d = nc.dram_tensor("Adst", [F, H], F32, kind="ExternalInput")
    bias_d = nc.dram_tensor("bias_rep", [128, F], F32, kind="ExternalInput")
    iorep_d = nc.dram_tensor("io_rep", [128, 128 * TPB], F16, kind="ExternalInput")
    idxlo_d = nc.dram_tensor("idx_lo", [128, NB * T_LO * 8], I16, kind="ExternalInput")
    idxhi_d = nc.dram_tensor("idx_hi", [128, NB * T_HI * 8], I16, kind="ExternalInput")
    selT_d = nc.dram_tensor("selT", [128, NTILES * 128], F8, kind="ExternalInput")
    dstloc_d = nc.dram_tensor("dstloc", [128, NTILES], F16, kind="ExternalInput")

    htab = nc.dram_tensor("htab", [NPAD, 256], F16)      # row: [p|a_src|h|pad]
    out_d = nc.dram_tensor("out", [OWN, F], F32, kind="ExternalOutput")

    with tile.TileContext(nc) as tc:
        with tc.tile_pool(name="const", bufs=1) as cp:
            W_t = cp.tile([F, F], F32)
            nc.sync.dma_start(out=W_t[:], in_=W_d[:])
            WT_t = cp.tile([F, F], F32)
            nc.sync.dma_start(out=WT_t[:], in_=WT_d[:])
            As_t = cp.tile([F, H], F32)
            nc.sync.dma_start(out=As_t[:], in_=As_d[:])
            Ad_t = cp.tile([F, H], F32)
            nc.sync.dma_start(out=Ad_t[:], in_=Ad_d[:])
            bias_t = cp.tile([128, F], F32)
            nc.sync.dma_start(out=bias_t[:], in_=bias_d[:])
            io_t = cp.tile([128, 128 * TPB], F16)
            nc.sync.dma_start(out=io_t[:], in_=iorep_d[:])
            idxlo_t = cp.tile([128, NB * T_LO * 8], I16)
            nc.sync.dma_start(out=idxlo_t[:], in_=idxlo_d[:])
            idxhi_t = cp.tile([128, NB * T_HI * 8], I16)
            nc.sync.dma_start(out=idxhi_t[:], in_=idxhi_d[:])
            dstloc_t = cp.tile([128, NTILES], F16)
            nc.sync.dma_start(out=dstloc_t[:], in_=dstloc_d[:])
            adall = cp.tile([128, NB, 4], F16)   # own-shard a_dst per block

            # Wcat = [W@Asrc | W | W@Adst]  -> proj psum: [a_src | h | a_dst]
            wcat = cp.tile([F, 136], F32)
            with tc.tile_pool(name="wps", bufs=1, space="PSUM") as wps:
                wa = wps.tile([F, 8], F32)
                nc.tensor.matmul(wa[:, 0:4], lhsT=WT_t[:], rhs=As_t[:],
                                 start=True, stop=True)
                nc.tensor.matmul(wa[:, 4:8], lhsT=WT_t[:], rhs=Ad_t[:],
                                 start=True, stop=True)
                nc.vector.tensor_copy(out=wcat[:, 0:4], in_=wa[:, 0:4])
                nc.vector.tensor_copy(out=wcat[:, 132:136], in_=wa[:, 4:8])
                nc.any.tensor_copy(out=wcat[:, 4:132], in_=W_t[:])
            wcat16 = cp.tile([F, 136], F16)
            nc.any.tensor_copy(out=wcat16[:], in_=wcat[:])

            # ---------------- phase 1: projection ----------------
            PB = 16
            with tc.tile_pool(name="p1", bufs=4) as p1, \
                 tc.tile_pool(name="p1ps", bufs=6, space="PSUM") as p1ps, \
                 tc.tile_pool(name="p1ps2", bufs=2, space="PSUM") as p1ps2:
                for t0 in range(0, NT, PB):
                    nb_t = min(PB, NT - t0)
                    xt = p1.tile([128, PB, 128], F16, tag="xt")
                    nc.sync.dma_start(
                        out=xt[:, 0:nb_t, :],
                        in_=xT[:, t0 * 128:(t0 + nb_t) * 128])
                    row = p1.tile([128, PB, 136], F16, tag="row")
                    for k in range(nb_t):
                        ps = p1ps.tile([128, 136], F32, tag="pps")
                        nc.tensor.matmul(ps[:], lhsT=xt[:, k, :], rhs=wcat16[:],
                                         start=True, stop=True)
                        nc.any.tensor_copy(out=row[:, k, 4:136], in_=ps[:, 0:132])
                    dst = htab[t0 * 128:(t0 + nb_t) * 128, 4:136].rearrange(
                        "(k p) c -> p k c", k=nb_t)
                    nc.sync.dma_start(out=dst, in_=row[:, 0:nb_t, 4:136])
                for t0 in range(0, NB, PB):
                    nb_t = min(PB, NB - t0)
                    xo = p1.tile([128, PB, 128], F16, tag="xt")
                    nc.sync.dma_start(
                        out=xo[:, 0:nb_t, :],
                        in_=xT_own[:, t0 * 128:(t0 + nb_t) * 128])
                    for k in range(nb_t):
                        ps2 = p1ps2.tile([128, 4], F32, tag="pps2")
                        nc.tensor.matmul(ps2[:], lhsT=xo[:, k, :],
                                         rhs=wcat16[:, 132:136],
                                         start=True, stop=True)
                        nc.any.tensor_copy(out=adall[:, t0 + k, :], in_=ps2[:])

            # ---------------- phase 2: edge processing ----------------
            if PHASE < 2:
                with tc.tile_pool(name="dbg", bufs=2) as dbg:
                    for q in range(NB):
                        z = dbg.tile([128, F], F16, tag="z")
                        nc.sync.dma_start(
                            out=z[:], in_=htab[q * 128:(q + 1) * 128, 0:128])
                        zo = dbg.tile([128, F], F32, tag="zo")
                        nc.any.tensor_copy(out=zo[:], in_=z[:])
                        nc.sync.dma_start(
                            out=out_d[q * 128:(q + 1) * 128, :], in_=zo[:])
            else:
              with tc.tile_pool(name="ep", bufs=3) as ep, \
                 tc.tile_pool(name="selp", bufs=4) as selp, \
                 tc.tile_pool(name="op", bufs=6) as op_, \
                 tc.tile_pool(name="eps", bufs=6, space="PSUM") as eps, \
                 tc.tile_pool(name="adps_p", bufs=2, space="PSUM") as adps_p:
                ngrp = -(-NB // G)
                for g in range(ngrp):
                    b0 = G * g
                    nb = min(G, NB - b0)
                    n_lo = nb * T_LO
                    n_hi = nb * T_HI
                    ntg = nb * TPB

                    # single_packet=True is the fast Q7 DGE path, but packets
                    # cap at 64 descriptors per engine -> at most 1024 idxs
                    # (8 tiles) per call.
                    def chunked_gather(out_tile, n_tiles, in_ap, idx_tile,
                                       slot0, elem):
                        done = 0
                        while done < n_tiles:
                            k = min(8, n_tiles - done)
                            nc.gpsimd.dma_gather(
                                out_ap=out_tile[:, done:done + k, :],
                                in_ap=in_ap,
                                idxs_ap=idx_tile[:, slot0 + done * 8:
                                                 slot0 + (done + k) * 8],
                                num_idxs=k * 128, num_idxs_reg=k * 128,
                                elem_size=elem, single_packet=True)
                            done += k

                    hlo = ep.tile([128, G * T_LO, 256], F16, tag="hlo")
                    chunked_gather(hlo, n_lo, htab[0:LOW, :], idxlo_t,
                                   b0 * T_LO * 8, 256)
                    hhi = ep.tile([128, G * T_HI, 256], F16, tag="hhi")
                    chunked_gather(hhi, n_hi, htab[LOW:NPAD, :], idxhi_t,
                                   b0 * T_HI * 8, 256)
                    selT_g = ep.tile([128, G * TPB, 128], F8, tag="selTg")
                    nc.sync.dma_start(
                        out=selT_g[:, 0:ntg, :],
                        in_=selT_d[:, b0 * TPB * 128:(b0 * TPB + ntg) * 128])

                    if PHASE < 3:
                        # consume gathers trivially
                        for q in range(nb):
                            zo = op_.tile([128, F], F32, tag="o")
                            nc.any.tensor_copy(
                                out=zo[:], in_=hlo[:, q * T_LO, 0:128])
                            nc.any.tensor_add(
                                out=zo[:], in0=zo[:],
                                in1=selT_g[:, q * TPB, 0:128])
                            nc.any.tensor_add(
                                out=zo[:], in0=zo[:],
                                in1=hhi[:, q * T_HI, 0:128])
                            nc.sync.dma_start(
                                out=out_d[(b0 + q) * 128:(b0 + q + 1) * 128, :],
                                in_=zo[:])
                        continue

                    # a_dst per edge: adps[e, (q,t,h)] = selT^T @ adall[block]
                    adps = adps_p.tile([128, G * TPB, 4], F32, tag="adps")
                    for tau in range(ntg):
                        nc.tensor.matmul(adps[:, tau, :],
                                         lhsT=selT_g[:, tau, :],
                                         rhs=adall[:, b0 + tau // TPB, :],
                                         start=True, stop=True)
                    # logits = a_src + a_dst  (block-major [q, tau, h] layout)
                    lg = ep.tile([128, G * TPB, 4], F32, tag="lg")
                    lgq = lg[:, 0:ntg, :].rearrange("p (q t) h -> p q t h", q=nb)
                    adq = adps[:, 0:ntg, :].rearrange("p (q t) h -> p q t h", q=nb)
                    asl = hlo[:, 0:n_lo, 4:8].rearrange("p (q t) h -> p q t h", q=nb)
                    ash = hhi[:, 0:n_hi, 4:8].rearrange("p (q t) h -> p q t h", q=nb)
                    nc.any.tensor_add(out=lgq[:, :, 0:T_LO, :], in0=asl,
                                      in1=adq[:, :, 0:T_LO, :])
                    nc.any.tensor_add(out=lgq[:, :, T_LO:TPB, :], in0=ash,
                                      in1=adq[:, :, T_LO:TPB, :])
                    # leaky relu: max(0.2*x, x)
                    lk = ep.tile([128, G * TPB * 4], F32, tag="lk")
                    lgf = lg[:, 0:ntg, :].rearrange("p t h -> p (t h)")
                    nc.vector.scalar_tensor_tensor(
                        out=lk[:, 0:ntg * 4], in0=lgf, scalar=NEG, in1=lgf,
                        op0=AOP.mult, op1=AOP.max)
                    # exp -> p written into gathered tiles' p-slots
                    lkq = lk[:, 0:ntg * 4].rearrange("p (q t h) -> p q t h",
                                                     q=nb, t=TPB)
                    plo = hlo[:, 0:n_lo, 0:4].rearrange("p (q t) h -> p q t h", q=nb)
                    phi = hhi[:, 0:n_hi, 0:4].rearrange("p (q t) h -> p q t h", q=nb)
                    nc.scalar.activation(out=plo, in_=lkq[:, :, 0:T_LO, :],
                                         func=ACT.Exp)
                    nc.scalar.activation(out=phi, in_=lkq[:, :, T_LO:TPB, :],
                                         func=ACT.Exp)
                    # messages: h *= p (broadcast over feature groups)
                    for hx, n_x in ((hlo, n_lo), (hhi, n_hi)):
                        hv = hx[:, 0:n_x, 8:136].rearrange(
                            "p t (h d) -> p t h d", h=4)
                        pv = hx[:, 0:n_x, 0:4].rearrange(
                            "p t (h o) -> p t h o", o=1).to_broadcast(
                            [128, n_x, 4, 32])
                        nc.any.tensor_mul(out=hv, in0=hv, in1=pv)

                    # per-block aggregation; sel for all TPB tiles of a
                    # block built in one 2x-mode tensor_tensor:
                    # sel[p, j, t] = (io_rep[p, j, t] == dstloc[p, t])
                    for q in range(nb):
                        Jb = (b0 + q) * TPB
                        sel = selp.tile([128, 128, TPB], F16, tag="sel")
                        dl = dstloc_t[:, Jb:Jb + TPB].rearrange(
                            "p (o t) -> p o t", o=1).to_broadcast([128, 128, TPB])
                        nc.vector.tensor_tensor(out=sel[:], in0=io_t[:].rearrange(
                            "p (j t) -> p j t", j=128), in1=dl, op=AOP.is_equal)
                        bps = eps.tile([128, 136], F32, tag="bps")
                        for tau in range(TPB):
                            if tau < T_LO:
                                rhs = hlo[:, q * T_LO + tau, 0:136]
                            else:
                                rhs = hhi[:, q * T_HI + (tau - T_LO), 0:136]
                            nc.tensor.matmul(bps[:], lhsT=sel[:, :, tau], rhs=rhs,
                                             start=(tau == 0),
                                             stop=(tau == TPB - 1))
                        # psum: [denom 0:4 | junk 4:8 | feats 8:136]
                        rcp = op_.tile([128, 4], F32, tag="rcp")
                        nc.vector.reciprocal(rcp[:], bps[:, 0:4])
                        o = op_.tile([128, F], F32, tag="o")
                        ov = o[:].rearrange("p (h d) -> p h d", h=H)
                        rv = rcp[:].rearrange("p (h o) -> p h o", o=1).to_broadcast(
                            [128, H, D])
                        nc.vector.tensor_tensor(
                            out=ov,
                            in0=bps[:, 8:136].rearrange("p (h d) -> p h d", h=H),
                            in1=rv, op=AOP.mult)
                        nc.vector.tensor_tensor(out=o[:], in0=o[:],
                                                in1=bias_t[:], op=AOP.add)
                        nc.scalar.activation(out=o[:], in_=o[:], func=ACT.Tanh)
                        nc.sync.dma_start(
                            out=out_d[(b0 + q) * 128:(b0 + q + 1) * 128, :],
                            in_=o[:])

    nc.compile()
    return nc


def prepped_in_maps(inputs):
    """test.py hook: per-core input maps for the cached program."""
    in_maps, T_LO, T_HI = _host_prep(
        inputs["x"], inputs["edge_index"], inputs["W"],
        inputs["att_src"], inputs["att_dst"], inputs["bias"])
    return in_maps


def cached_nc():
    """test.py hook: the (single) compiled program from the last kernel() call."""
    assert len(_CACHE) == 1, list(_CACHE)
    return next(iter(_CACHE.values()))


def kernel(**inputs):
    x = inputs["x"]
    edge_index = inputs["edge_index"]
    W = inputs["W"]
    att_src = inputs["att_src"]
    att_dst = inputs["att_dst"]
    bias = inputs["bias"]
    assert x.shape == (N, F) and edge_index.shape == (2, E_IN)

    from concourse import bass_utils

    in_maps, T_LO, T_HI = _host_prep(x, edge_index, W, att_src, att_dst, bias)
    key = (T_LO, T_HI)
    if key not in _CACHE:
        _CACHE[key] = _build_program(T_LO, T_HI)
    nc = _CACHE[key]
    res = bass_utils.run_bass_kernel_spmd(nc, in_maps, core_ids=list(range(NCORES)))
    out = np.concatenate([res.results[c]["out"] for c in range(NCORES)], axis=0)
    return np.ascontiguousarray(out[:N]).astype(np.float32)



# revision 10
# speedup vs baseline: 94.8975x; 1.3951x over previous
"""GAT message-passing kernel for 8 Trainium2 NeuronCores.

Strategy (dst-sharded graph parallel):
  - Nodes are padded to 50176 and split into 8 contiguous shards of 6272
    (49 blocks of 128 dst nodes per core).  Each core owns the edges whose
    destination falls in its shard (edges sorted by dst block on host).
  - Every core computes the full projection h = x @ W (plus per-node
    attention halves a_src/a_dst folded into the same matmul) and writes a
    per-node row table [p-slot | a_src | h] to its HBM.
  - Edge phase: per 128-edge tile, rows are fetched with a SWDGE dma_gather
    (512 B rows, int16 indices; the node table is split <32768 / >=32768 so
    indices fit int16). Per-edge a_dst comes from a PE matmul of a
    host-prepped one-hot selT tile (fp8 -- exact for 0/1 -- streamed
    contiguously from HBM) against an SBUF-resident f16 per-block a_dst
    table -- no second gather. Attention
    p = exp(leaky(a_src+a_dst)) is written into the gathered tile's p-slot,
    messages are scaled in place, and a one-hot selection matrix (built from
    iota==dst_local) is used as the stationary operand of a single PE matmul
    per tile that accumulates both the weighted messages and the softmax
    denominator in PSUM per 128-node block.
  - Host does index/layout prep only (edge sorting, padding, int16 index
    streams); all floating-point math runs on device.
"""

import sys

sys.path.insert(0, "/opt/trn_rl_repo")

import numpy as np

N = 50000
E_IN = 600000
F = 128          # feature dim (both in and out)
H = 4
D = 32
NEG = 0.2
NCORES = 8
BLK = 128
NB = 49                  # blocks per core
OWN = NB * BLK           # 6272
NPAD = NCORES * OWN      # 50176
NT = NPAD // 128         # 392 projection tiles
LOW = 32768              # src-index split for int16 gather indices
HI_ROWS = NPAD - LOW     # 17408
G = 4                    # blocks per gather group
PHASE = 3                # debug: 1=projection only, 2=+gathers, 3=full

_CACHE = {}


def _wrap_idx(flat):
    """int16 stream -> [128, len/16] SBUF layout (wrapped in 16 partitions,
    replicated across the 8 gpsimd cores)."""
    flat = np.ascontiguousarray(flat, dtype=np.int16)
    assert flat.size % 16 == 0
    w = flat.reshape(-1, 16).T              # [16, slots]
    out = np.empty((128, flat.size // 16), np.int16)
    for g in range(8):
        out[16 * g:16 * (g + 1)] = w
    return out


def _host_prep(x, edge_index, W, att_src, att_dst, bias):
    import ml_dtypes
    global _f8
    _f8 = ml_dtypes.float8_e4m3
    f16 = np.float16
    src0 = np.asarray(edge_index[0], dtype=np.int64)
    dst0 = np.asarray(edge_index[1], dtype=np.int64)
    loops = np.arange(NPAD, dtype=np.int64)
    src = np.concatenate([src0, loops])
    dst = np.concatenate([dst0, loops])

    core = dst // OWN
    blk = (dst % OWN) // BLK
    dl = (dst % BLK).astype(np.float32)
    is_hi = (src >= LOW).astype(np.int64)
    key = (core * NB + blk) * 2 + is_hi
    order = np.argsort(key, kind="stable")
    src_s = src[order]
    dl_s = dl[order]
    key_s = key[order]
    dst_own_s = (dst % OWN)[order]

    counts = np.bincount(key_s, minlength=NCORES * NB * 2)
    tiles = -(-counts // 128)
    T_LO = int(tiles[0::2].max())
    T_HI = int(tiles[1::2].max())
    TPB = T_LO + T_HI
    NTILES = NB * TPB

    starts = np.zeros(NCORES * NB * 2 + 1, np.int64)
    np.cumsum(counts, out=starts[1:])
    rank = np.arange(src_s.size, dtype=np.int64) - starts[key_s]
    c_s = key_s // (2 * NB)
    b_s = (key_s // 2) % NB
    hi_s = key_s & 1
    tau = np.where(hi_s == 0, rank // 128, T_LO + rank // 128)
    J = b_s * TPB + tau                      # tile index within core
    part = rank % 128

    # per-core padded streams
    idx_lo = np.zeros((NCORES, NB * T_LO * 128), np.int16)
    idx_hi = np.zeros((NCORES, NB * T_HI * 128), np.int16)
    dstloc = np.full((NCORES, 128, NTILES), -1.0, np.float16)
    selT = np.zeros((NCORES, 128, NTILES, 128), _f8)

    lo_m = hi_s == 0
    pos_lo = (b_s[lo_m] * T_LO + rank[lo_m] // 128) * 128 + part[lo_m]
    idx_lo[c_s[lo_m], pos_lo] = src_s[lo_m].astype(np.int16)
    hi_m = ~lo_m
    pos_hi = (b_s[hi_m] * T_HI + rank[hi_m] // 128) * 128 + part[hi_m]
    idx_hi[c_s[hi_m], pos_hi] = (src_s[hi_m] - LOW).astype(np.int16)
    dstloc[c_s, part, J] = dl_s
    selT[c_s, dl_s.astype(np.int64), J, part] = 1.0

    # shared constants
    xT = np.zeros((F, NPAD), np.float16)
    xT[:, :N] = np.asarray(x, np.float32).T.astype(np.float16)
    Wf = np.ascontiguousarray(np.asarray(W, np.float32))
    WT = np.ascontiguousarray(Wf.T)
    Asrc = np.zeros((F, H), np.float32)
    Adst = np.zeros((F, H), np.float32)
    for hh in range(H):
        Asrc[hh * D:(hh + 1) * D, hh] = np.asarray(att_src, np.float32)[hh]
        Adst[hh * D:(hh + 1) * D, hh] = np.asarray(att_dst, np.float32)[hh]
    bias_rep = np.ascontiguousarray(
        np.broadcast_to(np.asarray(bias, np.float32), (128, F)))
    io_rep = np.ascontiguousarray(np.broadcast_to(
        np.repeat(np.arange(128, dtype=f16), TPB)[None, :], (128, 128 * TPB)))

    in_maps = []
    for c in range(NCORES):
        in_maps.append({
            "xT": xT,
            "xT_own": np.ascontiguousarray(xT[:, c * OWN:(c + 1) * OWN]),
            "W": Wf,
            "WT": WT,
            "Asrc": Asrc,
            "Adst": Adst,
            "bias_rep": bias_rep,
            "io_rep": io_rep,
            "idx_lo": _wrap_idx(idx_lo[c]),
            "idx_hi": _wrap_idx(idx_hi[c]),
            "selT": np.ascontiguousarray(selT[c].reshape(128, NTILES * 128)),
            "dstloc": dstloc[c],
        })
    return in_maps, T_LO, T_HI


def _build_program(T_LO, T_HI):
    import concourse.bacc as bacc
    import concourse.mybir as mybir
    import concourse.tile as tile

    F16 = mybir.dt.float16
    F32 = mybir.dt.float32
    F8 = mybir.dt.float8e4
    I16 = mybir.dt.int16
    AOP = mybir.AluOpType
    ACT = mybir.ActivationFunctionType

    TPB = T_LO + T_HI
    NTILES = NB * TPB

    nc = bacc.Bacc("TRN2", target_bir_lowering=False)

    xT = nc.dram_tensor("xT", [F, NPAD], F16, kind="ExternalInput")
    xT_own = nc.dram_tensor("xT_own", [F, OWN], F16, kind="ExternalInput")
    W_d = nc.dram_tensor("W", [F, F], F32, kind="ExternalInput")
    WT_d = nc.dram_tensor("WT", [F, F], F32, kind="ExternalInput")
    As_d = nc.dram_tensor("Asrc", [F, H], F32, kind="ExternalInput")
    Ad_d = nc.dram_tensor("Adst", [F, H], F32, kind="ExternalInput")
    bias_d = nc.dram_tensor("bias_rep", [128, F], F32, kind="ExternalInput")
    iorep_d = nc.dram_tensor("io_rep", [128, 128 * TPB], F16, kind="ExternalInput")
    idxlo_d = nc.dram_tensor("idx_lo", [128, NB * T_LO * 8], I16, kind="ExternalInput")
    idxhi_d = nc.dram_tensor("idx_hi", [128, NB * T_HI * 8], I16, kind="ExternalInput")
    selT_d = nc.dram_tensor("selT", [128, NTILES * 128], F8, kind="ExternalInput")
    dstloc_d = nc.dram_tensor("dstloc", [128, NTILES], F16, kind="ExternalInput")

    htab = nc.dram_tensor("htab", [NPAD, 256], F16)      # row: [p|a_src|h|pad]
    out_d = nc.dram_tensor("out", [OWN, F], F16, kind="ExternalOutput")
    htab_writes = []          # projection DMAs into htab (race-fix deps)

    with tile.TileContext(nc) as tc:
        with tc.tile_pool(name="const", bufs=1) as cp:
            W_t = cp.tile([F, F], F32)
            nc.sync.dma_start(out=W_t[:], in_=W_d[:])
            WT_t = cp.tile([F, F], F32)
            nc.sync.dma_start(out=WT_t[:], in_=WT_d[:])
            As_t = cp.tile([F, H], F32)
            nc.sync.dma_start(out=As_t[:], in_=As_d[:])
            Ad_t = cp.tile([F, H], F32)
            nc.sync.dma_start(out=Ad_t[:], in_=Ad_d[:])
            bias_t = cp.tile([128, F], F32)
            nc.sync.dma_start(out=bias_t[:], in_=bias_d[:])
            io_t = cp.tile([128, 128 * TPB], F16)
            nc.sync.dma_start(out=io_t[:], in_=iorep_d[:])
            idxlo_t = cp.tile([128, NB * T_LO * 8], I16)
            nc.sync.dma_start(out=idxlo_t[:], in_=idxlo_d[:])
            idxhi_t = cp.tile([128, NB * T_HI * 8], I16)
            nc.sync.dma_start(out=idxhi_t[:], in_=idxhi_d[:])
            dstloc_t = cp.tile([128, NTILES], F16)
            nc.sync.dma_start(out=dstloc_t[:], in_=dstloc_d[:])
            adall = cp.tile([128, NB, 4], F16)   # own-shard a_dst per block

            # Wcat = [W@Asrc | W | W@Adst]  -> proj psum: [a_src | h | a_dst]
            wcat = cp.tile([F, 136], F32)
            with tc.tile_pool(name="wps", bufs=1, space="PSUM") as wps:
                wa = wps.tile([F, 8], F32)
                nc.tensor.matmul(wa[:, 0:4], lhsT=WT_t[:], rhs=As_t[:],
                                 start=True, stop=True)
                nc.tensor.matmul(wa[:, 4:8], lhsT=WT_t[:], rhs=Ad_t[:],
                                 start=True, stop=True)
                nc.vector.tensor_copy(out=wcat[:, 0:4], in_=wa[:, 0:4])
                nc.vector.tensor_copy(out=wcat[:, 132:136], in_=wa[:, 4:8])
                nc.any.tensor_copy(out=wcat[:, 4:132], in_=W_t[:])
            wcat16 = cp.tile([F, 136], F16)
            nc.any.tensor_copy(out=wcat16[:], in_=wcat[:])

            # ---------------- phase 1: projection ----------------
            PB = 16
            with tc.tile_pool(name="p1", bufs=4) as p1, \
                 tc.tile_pool(name="p1ps", bufs=6, space="PSUM") as p1ps, \
                 tc.tile_pool(name="p1ps2", bufs=2, space="PSUM") as p1ps2:
                for t0 in range(0, NT, PB):
                    nb_t = min(PB, NT - t0)
                    xt = p1.tile([128, PB, 128], F16, tag="xt")
                    nc.sync.dma_start(
                        out=xt[:, 0:nb_t, :],
                        in_=xT[:, t0 * 128:(t0 + nb_t) * 128])
                    row = p1.tile([128, PB, 136], F16, tag="row")
                    for k in range(nb_t):
                        ps = p1ps.tile([128, 136], F32, tag="pps")
                        nc.tensor.matmul(ps[:], lhsT=xt[:, k, :], rhs=wcat16[:],
                                         start=True, stop=True)
                        nc.any.tensor_copy(out=row[:, k, 4:136], in_=ps[:, 0:132])
                    dst = htab[t0 * 128:(t0 + nb_t) * 128, 4:136].rearrange(
                        "(k p) c -> p k c", k=nb_t)
                    htab_writes.append(
                        nc.sync.dma_start(out=dst, in_=row[:, 0:nb_t, 4:136]))
                for t0 in range(0, NB, PB):
                    nb_t = min(PB, NB - t0)
                    xo = p1.tile([128, PB, 128], F16, tag="xt")
                    nc.sync.dma_start(
                        out=xo[:, 0:nb_t, :],
                        in_=xT_own[:, t0 * 128:(t0 + nb_t) * 128])
                    for k in range(nb_t):
                        ps2 = p1ps2.tile([128, 4], F32, tag="pps2")
                        nc.tensor.matmul(ps2[:], lhsT=xo[:, k, :],
                                         rhs=wcat16[:, 132:136],
                                         start=True, stop=True)
                        nc.any.tensor_copy(out=adall[:, t0 + k, :], in_=ps2[:])

            # ---------------- phase 2: edge processing ----------------
            if PHASE < 2:
                with tc.tile_pool(name="dbg", bufs=2) as dbg:
                    for q in range(NB):
                        z = dbg.tile([128, F], F16, tag="z")
                        nc.sync.dma_start(
                            out=z[:], in_=htab[q * 128:(q + 1) * 128, 0:128])
                        zo = dbg.tile([128, F], F16, tag="zo")
                        nc.any.tensor_copy(out=zo[:], in_=z[:])
                        nc.sync.dma_start(
                            out=out_d[q * 128:(q + 1) * 128, :], in_=zo[:])
            else:
              with tc.tile_pool(name="ep", bufs=3) as ep, \
                 tc.tile_pool(name="selp", bufs=4) as selp, \
                 tc.tile_pool(name="op", bufs=6) as op_, \
                 tc.tile_pool(name="eps", bufs=6, space="PSUM") as eps, \
                 tc.tile_pool(name="adps_p", bufs=2, space="PSUM") as adps_p:
                ngrp = -(-NB // G)
                for g in range(ngrp):
                    b0 = G * g
                    nb = min(G, NB - b0)
                    n_lo = nb * T_LO
                    n_hi = nb * T_HI
                    ntg = nb * TPB

                    # One multi-packet SWDGE call per (group, lo/hi): the
                    # ~1 us fixed Q7 cost per call dominates at 1024-idx
                    # granularity, so batch all of a group's tiles into a
                    # single descriptor-generation pass.
                    def chunked_gather(out_tile, n_tiles, in_ap, idx_tile,
                                       slot0, elem):
                        g = nc.gpsimd.dma_gather(
                            out_ap=out_tile[:, 0:n_tiles, :],
                            in_ap=in_ap,
                            idxs_ap=idx_tile[:, slot0:slot0 + n_tiles * 8],
                            num_idxs=n_tiles * 128,
                            num_idxs_reg=n_tiles * 128,
                            elem_size=elem, single_packet=False)
                        for w in htab_writes:
                            tile.add_dep_helper(
                                getattr(g, "ins", g), getattr(w, "ins", w),
                                reason="gather reads htab after projection")

                    hlo = ep.tile([128, G * T_LO, 256], F16, tag="hlo")
                    chunked_gather(hlo, n_lo, htab[0:LOW, :], idxlo_t,
                                   b0 * T_LO * 8, 256)
                    hhi = ep.tile([128, G * T_HI, 256], F16, tag="hhi")
                    chunked_gather(hhi, n_hi, htab[LOW:NPAD, :], idxhi_t,
                                   b0 * T_HI * 8, 256)
                    selT_g = ep.tile([128, G * TPB, 128], F8, tag="selTg")
                    nc.sync.dma_start(
                        out=selT_g[:, 0:ntg, :],
                        in_=selT_d[:, b0 * TPB * 128:(b0 * TPB + ntg) * 128])

                    if PHASE < 3:
                        # consume gathers trivially
                        for q in range(nb):
                            zo = op_.tile([128, F], F16, tag="zdbg")
                            nc.any.tensor_copy(
                                out=zo[:], in_=hlo[:, q * T_LO, 0:128])
                            nc.any.tensor_add(
                                out=zo[:], in0=zo[:],
                                in1=selT_g[:, q * TPB, 0:128])
                            nc.any.tensor_add(
                                out=zo[:], in0=zo[:],
                                in1=hhi[:, q * T_HI, 0:128])
                            nc.sync.dma_start(
                                out=out_d[(b0 + q) * 128:(b0 + q + 1) * 128, :],
                                in_=zo[:])
                        continue

                    # a_dst per edge: adps[e, (q,t,h)] = selT^T @ adall[block]
                    adps = adps_p.tile([128, G * TPB, 4], F32, tag="adps")
                    for tau in range(ntg):
                        nc.tensor.matmul(adps[:, tau, :],
                                         lhsT=selT_g[:, tau, :],
                                         rhs=adall[:, b0 + tau // TPB, :],
                                         start=True, stop=True)
                    # logits = a_src + a_dst  (block-major [q, tau, h] layout)
                    lg = ep.tile([128, G * TPB, 4], F32, tag="lg")
                    lgq = lg[:, 0:ntg, :].rearrange("p (q t) h -> p q t h", q=nb)
                    adq = adps[:, 0:ntg, :].rearrange("p (q t) h -> p q t h", q=nb)
                    asl = hlo[:, 0:n_lo, 4:8].rearrange("p (q t) h -> p q t h", q=nb)
                    ash = hhi[:, 0:n_hi, 4:8].rearrange("p (q t) h -> p q t h", q=nb)
                    nc.any.tensor_add(out=lgq[:, :, 0:T_LO, :], in0=asl,
                                      in1=adq[:, :, 0:T_LO, :])
                    nc.any.tensor_add(out=lgq[:, :, T_LO:TPB, :], in0=ash,
                                      in1=adq[:, :, T_LO:TPB, :])
                    # leaky relu: max(0.2*x, x)
                    lk = ep.tile([128, G * TPB * 4], F32, tag="lk")
                    lgf = lg[:, 0:ntg, :].rearrange("p t h -> p (t h)")
                    nc.vector.scalar_tensor_tensor(
                        out=lk[:, 0:ntg * 4], in0=lgf, scalar=NEG, in1=lgf,
                        op0=AOP.mult, op1=AOP.max)
                    # exp -> p written into gathered tiles' p-slots
                    lkq = lk[:, 0:ntg * 4].rearrange("p (q t h) -> p q t h",
                                                     q=nb, t=TPB)
                    plo = hlo[:, 0:n_lo, 0:4].rearrange("p (q t) h -> p q t h", q=nb)
                    phi = hhi[:, 0:n_hi, 0:4].rearrange("p (q t) h -> p q t h", q=nb)
                    nc.scalar.activation(out=plo, in_=lkq[:, :, 0:T_LO, :],
                                         func=ACT.Exp)
                    nc.scalar.activation(out=phi, in_=lkq[:, :, T_LO:TPB, :],
                                         func=ACT.Exp)
                    # messages: h *= p (broadcast over feature groups)
                    for hx, n_x in ((hlo, n_lo), (hhi, n_hi)):
                        hv = hx[:, 0:n_x, 8:136].rearrange(
                            "p t (h d) -> p t h d", h=4)
                        pv = hx[:, 0:n_x, 0:4].rearrange(
                            "p t (h o) -> p t h o", o=1).to_broadcast(
                            [128, n_x, 4, 32])
                        nc.any.tensor_mul(out=hv, in0=hv, in1=pv)

                    # per-block aggregation; sel for all TPB tiles of a
                    # block built in one 2x-mode tensor_tensor:
                    # sel[p, j, t] = (io_rep[p, j, t] == dstloc[p, t])
                    for q in range(nb):
                        Jb = (b0 + q) * TPB
                        sel = selp.tile([128, 128, TPB], F16, tag="sel")
                        dl = dstloc_t[:, Jb:Jb + TPB].rearrange(
                            "p (o t) -> p o t", o=1).to_broadcast([128, 128, TPB])
                        nc.vector.tensor_tensor(out=sel[:], in0=io_t[:].rearrange(
                            "p (j t) -> p j t", j=128), in1=dl, op=AOP.is_equal)
                        bps = eps.tile([128, 136], F32, tag="bps")
                        for tau in range(TPB):
                            if tau < T_LO:
                                rhs = hlo[:, q * T_LO + tau, 0:136]
                            else:
                                rhs = hhi[:, q * T_HI + (tau - T_LO), 0:136]
                            nc.tensor.matmul(bps[:], lhsT=sel[:, :, tau], rhs=rhs,
                                             start=(tau == 0),
                                             stop=(tau == TPB - 1))
                        # psum: [denom 0:4 | junk 4:8 | feats 8:136]
                        rcp = op_.tile([128, 4], F32, tag="rcp")
                        nc.vector.reciprocal(rcp[:], bps[:, 0:4])
                        o = op_.tile([128, F], F32, tag="o")
                        ov = o[:].rearrange("p (h d) -> p h d", h=H)
                        rv = rcp[:].rearrange("p (h o) -> p h o", o=1).to_broadcast(
                            [128, H, D])
                        nc.vector.tensor_tensor(
                            out=ov,
                            in0=bps[:, 8:136].rearrange("p (h d) -> p h d", h=H),
                            in1=rv, op=AOP.mult)
                        nc.vector.tensor_tensor(out=o[:], in0=o[:],
                                                in1=bias_t[:], op=AOP.add)
                        o16 = op_.tile([128, F], F16, tag="o16")
                        nc.scalar.activation(out=o16[:], in_=o[:], func=ACT.Tanh)
                        nc.sync.dma_start(
                            out=out_d[(b0 + q) * 128:(b0 + q + 1) * 128, :],
                            in_=o16[:])

    nc.compile()
    return nc


def prepped_in_maps(inputs):
    """test.py hook: per-core input maps for the cached program."""
    in_maps, T_LO, T_HI = _host_prep(
        inputs["x"], inputs["edge_index"], inputs["W"],
        inputs["att_src"], inputs["att_dst"], inputs["bias"])
    return in_maps


def cached_nc():
    """test.py hook: the (single) compiled program from the last kernel() call."""
    assert len(_CACHE) == 1, list(_CACHE)
    return next(iter(_CACHE.values()))


def kernel(**inputs):
    x = inputs["x"]
    edge_index = inputs["edge_index"]
    W = inputs["W"]
    att_src = inputs["att_src"]
    att_dst = inputs["att_dst"]
    bias = inputs["bias"]
    assert x.shape == (N, F) and edge_index.shape == (2, E_IN)

    from concourse import bass_utils

    in_maps, T_LO, T_HI = _host_prep(x, edge_index, W, att_src, att_dst, bias)
    key = (T_LO, T_HI)
    if key not in _CACHE:
        _CACHE[key] = _build_program(T_LO, T_HI)
    nc = _CACHE[key]
    res = bass_utils.run_bass_kernel_spmd(nc, in_maps, core_ids=list(range(NCORES)))
    out = np.concatenate([res.results[c]["out"] for c in range(NCORES)], axis=0)
    return np.ascontiguousarray(out[:N]).astype(np.float32)

